# revision 24
# baseline (speedup 1.0000x reference)
"""Multi-head attention (S=2048, B=2, D=1024, H=16, Hd=64) on 8 trn2 cores.

Sharding: core = (batch b, head-group g of 4 heads) -> 2*4 = 8 cores.
Each core computes full attention for its 4 heads / 1 batch and a partial
output projection (row-parallel Wo); the host sums the 4 partials per batch
and adds bo.

v2 design (vs the 490us baseline):
  - 8 attention rounds of (head-pair p, 512-col s-quarter): score psum
    pipeline 3 deep (3x2 banks) + 2 chain accumulators = 8 banks.
  - scores row-group packed (2 heads concurrent, K=64 at rows 0/64).
  - exp split across engines: most tiles exact on ACT (exp with bias
    ln(C_EFF) to match scales), a few per round on DVE via a 2-term
    Schraudolph bit-trick exp (f32->int16 tensor_scalar passes, bitcast to
    bf16, summed on GpSimd) -- relative error ~ +-1.5%, softmax-normalized.
  - chains drained psum->sbuf by ACT right after the last accumulate
    (frees psum for the next round); Z row kept f32; DRAM-bounce partition
    broadcast of Z off the critical path; DVE reciprocal + normalize mults.
  - attn2 / Wo in bf16; K-outer projections start on first DMA'd x tile;
    out-projection bursts after each s-quarter completes both pairs.
"""

import sys

for _p in ("/opt/trn_rl_repo", "/root/.axon_site/_ro/trn_rl_repo"):
    if _p not in sys.path:
        sys.path.insert(0, _p)

import numpy as np
import ml_dtypes

S = 2048
B = 2
D = 1024
H = 16
HD = 64
NH = 4   # heads per core
P = 128
KD = D // P   # 8 contraction tiles for projections
NT = S // P   # 16 t (key) tiles
NQ = 4        # s-quarters per pair
QW = S // NQ  # 512 columns per quarter

BF16 = ml_dtypes.bfloat16

# Schraudolph 2-term exp constants (see accuracy sim):
#   S(y,d) = bitcast_bf16(int16(rint(128*y + 16256 + d)))
#   S(y,-80.25) + S(y,-142.75) ~= C_EFF * 2^y, max rel dev ~1.45%
# with y = score * 0.125 * log2(e).  ACT tiles use exp(0.125*x + ln(C_EFF))
# so both shares carry the same scale; softmax normalization removes it.
LOG2E = 1.4426950408889634
TS_MULT = 0.125 * 128 * LOG2E        # 23.083120654223414
TS_OFF1 = 16256.0 - 80.25
TS_OFF2 = 16256.0 - 142.75
ACT_BIAS = 0.1396463666  # ln(1.1498665502852918)

# which exp units (t index 0..15) go to the DVE path, per round
DVE_T = (3, 8, 13)

_BUILD_CACHE = {}


def build_bass(s=S, dve_t=DVE_T, debug_taps=False):
    import concourse.bacc as bacc
    import concourse.bass as bass
    import concourse.mybir as mybir
    import concourse.tile as tile

    f32 = mybir.dt.float32
    bf16 = mybir.dt.bfloat16
    i16 = mybir.dt.int16
    AF = mybir.ActivationFunctionType
    ALU = mybir.AluOpType

    nt = s // P
    nq = s // QW

    nc = bacc.Bacc("TRN2", target_bir_lowering=False, debug=False, num_devices=8)

    xq = nc.dram_tensor("xq_t", [D, s], bf16, kind="ExternalInput").ap()
    xk = nc.dram_tensor("xk_t", [D, s], bf16, kind="ExternalInput").ap()
    xv = nc.dram_tensor("xv_t", [D, s], bf16, kind="ExternalInput").ap()
    wq = nc.dram_tensor("wq_t", [D, 256], bf16, kind="ExternalInput").ap()
    wk = nc.dram_tensor("wk_t", [D, 256], bf16, kind="ExternalInput").ap()
    wv = nc.dram_tensor("wv_t", [D, 256], bf16, kind="ExternalInput").ap()
    wo = nc.dram_tensor("wo_h", [P, 2, D], bf16, kind="ExternalInput").ap()
    bq2 = nc.dram_tensor("bq2", [P, 2], f32, kind="ExternalInput").ap()
    bk2 = nc.dram_tensor("bk2", [P, 2], f32, kind="ExternalInput").ap()
    bv4 = nc.dram_tensor("bv4", [P, 256], f32, kind="ExternalInput").ap()
    out = nc.dram_tensor("out", [s, D], f32, kind="ExternalOutput").ap()

    from contextlib import ExitStack

    with tile.TileContext(nc) as tc, ExitStack() as ctx:
        consts = ctx.enter_context(tc.tile_pool(name="consts", bufs=1))
        persist = ctx.enter_context(tc.tile_pool(name="persist", bufs=1))
        xqpool = ctx.enter_context(tc.tile_pool(name="xqpool", bufs=8))
        xvpool = ctx.enter_context(tc.tile_pool(name="xvpool", bufs=8))
        drampool = ctx.enter_context(tc.tile_pool(name="drampool", bufs=2, space="DRAM"))

        # ---- constants + x loads, in consumption order (K, Q, V) ------
        wk_sb = consts.tile([P, KD, 256], bf16, name="wk_sb")
        nc.sync.dma_start(out=wk_sb, in_=wk.rearrange("(k p) e -> p k e", p=P))
        bk_sb = consts.tile([P, 2], f32, name="bk_sb")
        nc.sync.dma_start(out=bk_sb, in_=bk2)

        # ---- persistent activations -----------------------------------
        q2 = persist.tile([P, 2, s], bf16, name="q2")
        k2 = persist.tile([P, 2, s], bf16, name="k2")
        v_aug = persist.tile([P, NH, nt, 65], bf16, name="v_aug")
        nc.vector.memset(v_aug, 1.0)  # col 64 = ones column -> row 64 is Z
        attn2 = persist.tile([P, 2, s], bf16, name="attn2")

        def load_x(pool, xdram, tag):
            x3 = xdram.rearrange("(k p) s -> k p s", p=P)
            tiles = []
            for k in range(KD):
                xt = pool.tile([P, s], bf16, tag="x", name=f"{tag}{k}")
                nc.sync.dma_start(out=xt, in_=x3[k])
                tiles.append(xt)
            return tiles

        def load_xq_qtr(qtr):
            # xq column-quarter: 8 sub-tiles [P, QW] (1 MB total) so the
            # first attention round is not gated on the full 4 MB of xq
            x3 = xq.rearrange("(k p) s -> k p s", p=P)
            tiles = []
            for k in range(KD):
                xt = xqpool.tile([P, QW], bf16, tag="xq", name=f"xq{qtr}_{k}")
                nc.sync.dma_start(out=xt, in_=x3[k, :, qtr * QW:(qtr + 1) * QW])
                tiles.append(xt)
            return tiles

        # K projection in its own scoped pools so both the xk SBUF and the
        # psum banks recycle for the attention machinery below.
        with tc.tile_pool(name="xkpool", bufs=8) as xkpool:
            xk_t = load_x(xkpool, xk, "xk")

            wq_sb = consts.tile([P, KD, 256], bf16, name="wq_sb")
            nc.sync.dma_start(out=wq_sb, in_=wq.rearrange("(k p) e -> p k e", p=P))
            bq_sb = consts.tile([P, 2], f32, name="bq_sb")
            nc.sync.dma_start(out=bq_sb, in_=bq2)
            xq_q = [load_xq_qtr(0)]

            wv_sb = consts.tile([P, KD, 256], bf16, name="wv_sb")
            nc.sync.dma_start(out=wv_sb, in_=wv.rearrange("(k p) e -> p k e", p=P))
            bv_sb = consts.tile([P, 256], f32, name="bv_sb")
            nc.sync.dma_start(out=bv_sb, in_=bv4)
            xv_t = load_x(xvpool, xv, "xv")

            for qtr in range(1, NQ):
                xq_q.append(load_xq_qtr(qtr))

            wo_sb = consts.tile([P, 2, D], bf16, name="wo_sb")
            nc.sync.dma_start(out=wo_sb, in_=wo)
            actb = consts.tile([P, 1], f32, name="actb")
            nc.vector.memset(actb, ACT_BIAS)

            with tc.tile_pool(name="ppk", bufs=4, space="PSUM") as ppk:
                # K: 4 accumulators (p, sh), k-outer over xk tiles
                kps = {}
                for p in range(2):
                    for sh in range(2):
                        kps[(p, sh)] = ppk.tile([P, s // 2], f32, tag="qk",
                                                name=f"kps{p}{sh}")
                for k in range(KD):
                    for p in range(2):
                        for sh in range(2):
                            for c in range(2):
                                nc.tensor.matmul(
                                    kps[(p, sh)][:, c * QW:(c + 1) * QW],
                                    lhsT=wk_sb[:, k, p * P:(p + 1) * P],
                                    rhs=xk_t[k][:, sh * (s // 2) + c * QW:
                                                sh * (s // 2) + (c + 1) * QW],
                                    start=(k == 0),
                                    stop=(k == KD - 1),
                                )
                for p in range(2):
                    for sh in range(2):
                        nc.vector.tensor_scalar(
                            k2[:, p, sh * (s // 2):(sh + 1) * (s // 2)],
                            kps[(p, sh)], bk_sb[:, p:p + 1],
                            None, ALU.add,
                        )

        # ---- attention rounds with V/Q/out-proj work as PE fillers ----
        epool = ctx.enter_context(tc.tile_pool(name="epool", bufs=18))
        tpool = ctx.enter_context(tc.tile_pool(name="tpool", bufs=6))
        cdpool = ctx.enter_context(tc.tile_pool(name="cdpool", bufs=4))
        zpool = ctx.enter_context(tc.tile_pool(name="zpool", bufs=2))
        ospool = ctx.enter_context(tc.tile_pool(name="ospool", bufs=3))
        wide = ctx.enter_context(tc.tile_pool(name="wide", bufs=2, space="PSUM"))
        accp = ctx.enter_context(tc.tile_pool(name="accp", bufs=4, space="PSUM"))

        def v_lams(t):
            # V proj for key-tile t as two filler lambdas; vps time-shares
            # the accp banks (chains are allocated lazily at first attnV)
            st = {}

            def f1():
                st["ps"] = accp.tile([P, 256], f32, tag="chain", name=f"vps{t}")
                for k in range(4):
                    nc.tensor.matmul(
                        st["ps"],
                        lhsT=xv_t[k][:, t * P:(t + 1) * P],
                        rhs=wv_sb[:, k, :],
                        start=(k == 0), stop=False,
                    )

            def f2():
                for k in range(4, KD):
                    nc.tensor.matmul(
                        st["ps"],
                        lhsT=xv_t[k][:, t * P:(t + 1) * P],
                        rhs=wv_sb[:, k, :],
                        start=False, stop=(k == KD - 1),
                    )
                nc.vector.tensor_tensor(
                    v_aug[:, :, t, 0:64],
                    st["ps"].rearrange("p (h d) -> p h d", h=NH),
                    bv_sb.rearrange("p (h d) -> p h d", h=NH),
                    ALU.add,
                )

            return [("v1", t, f1), ("v2", t, f2)]

        def qq_lams(psel, qtr):
            # Q proj for (pair, s-quarter): 8 MMs into one psum bank
            st = {}

            def f1():
                st["ps"] = accp.tile([P, QW], f32, tag="chain",
                                     name=f"qc{psel}{qtr}")
                for k in range(4):
                    nc.tensor.matmul(
                        st["ps"],
                        lhsT=wq_sb[:, k, psel * P:(psel + 1) * P],
                        rhs=xq_q[qtr][k],
                        start=(k == 0), stop=False,
                    )

            def f2():
                for k in range(4, KD):
                    nc.tensor.matmul(
                        st["ps"],
                        lhsT=wq_sb[:, k, psel * P:(psel + 1) * P],
                        rhs=xq_q[qtr][k],
                        start=False, stop=(k == KD - 1),
                    )
                nc.vector.tensor_scalar(
                    q2[:, psel, qtr * QW:(qtr + 1) * QW],
                    st["ps"], bq_sb[:, psel:psel + 1],
                    None, ALU.add,
                )

            return [("q1", (psel, qtr), f1), ("q2", (psel, qtr), f2)]

        def exp_act(et, sc):
            nc.scalar.activation(et, sc, AF.Exp, bias=actb, scale=0.125)

        def exp_dve(et, sc):
            # half-tile (per-head) passes to cut latency; pair-adds on gpsimd
            t1 = tpool.tile([P, QW * 2], i16, tag="ts", name="t1")
            t2 = tpool.tile([P, QW * 2], i16, tag="ts", name="t2")
            b1 = t1.bitcast(bf16)
            b2 = t2.bitcast(bf16)
            for c in range(2):
                h = slice(c * QW, (c + 1) * QW)
                nc.vector.tensor_scalar(t1[:, h], sc[:, h], TS_MULT, TS_OFF1,
                                        ALU.mult, ALU.add)
                nc.vector.tensor_scalar(t2[:, h], sc[:, h], TS_MULT, TS_OFF2,
                                        ALU.mult, ALU.add)
                nc.gpsimd.tensor_tensor(et[:, h], b1[:, h], b2[:, h], ALU.add)

        def op_lams(sc_i):
            st = {}

            def f1():
                st["op"] = wide.tile([P, D], f32, tag="wide", name=f"op{sc_i}")
                for nh_i in range(2):
                    for p in range(2):
                        nc.tensor.matmul(
                            st["op"][:, nh_i * 512:(nh_i + 1) * 512],
                            lhsT=attn2[:, p, sc_i * P:(sc_i + 1) * P],
                            rhs=wo_sb[:, p, nh_i * 512:(nh_i + 1) * 512],
                            start=(p == 0),
                            stop=(p == 1),
                        )

            def f2():
                ob = ospool.tile([P, D], f32, tag="ob", name="ob")
                nc.vector.tensor_copy(ob, st["op"])
                nc.sync.dma_start(out=out[sc_i * P:(sc_i + 1) * P, :], in_=ob)

            return [("op1", sc_i, f1), ("op2", sc_i, f2)]

        blks = s // P // nq

        def burst_lams(q):
            lams = []
            for blk in range(blks):
                lams += op_lams(q * blks + blk)
            return lams

        def attn_round(p, q, round_dve_t, carry, fill_start=0):
            """Emit scores+exp for round (p,q), popping deferred work from
            `carry` between slots.  Returns this round's own deferred attnV
            + normalize lambdas (to be carried into the next round)."""
            cols = slice(q * QW, (q + 1) * QW)
            heads = (2 * p, 2 * p + 1)
            ets = {}
            for t in range(nt):
                sc = wide.tile([P, 2 * QW], f32, tag="wide", name="sc")
                for hi in range(2):
                    rlo = 64 * hi
                    nc.tensor.matmul(
                        sc[:, hi * QW:(hi + 1) * QW],
                        lhsT=k2[rlo:rlo + 64, p, t * P:(t + 1) * P],
                        rhs=q2[rlo:rlo + 64, p, cols],
                        start=True,
                        stop=True,
                        tile_position=(rlo, 0),
                    )
                et = epool.tile([P, 2 * QW], bf16, tag="exp", name="et")
                ets[t] = et
                if t in round_dve_t:
                    exp_dve(et, sc)
                else:
                    exp_act(et, sc)
                if t >= fill_start and carry:
                    n = -(-len(carry) // (nt - t))
                    for _ in range(min(n, len(carry))):
                        carry.pop(0)[2]()

            # deferred attnV accumulation + normalize, run inside later rounds
            st = {}
            lams = []
            for t in range(nt):
                def av(t=t):
                    if t == 0:
                        st["ch"] = [
                            accp.tile([P, QW], f32, tag="chain", name=f"ch{hi}")
                            for hi in range(2)
                        ]
                    for hi in range(2):
                        nc.tensor.matmul(
                            st["ch"][hi][0:65, :],
                            lhsT=v_aug[:, heads[hi], t, :],
                            rhs=ets[t][:, hi * QW:(hi + 1) * QW],
                            start=(t == 0),
                            stop=(t == nt - 1),
                        )
                lams.append(("av", (p, q, t), av))

            def n1():
                st["zrow"] = zpool.tile([1, 2 * QW], f32, tag="zrow", name="zrow")
                st["cd"] = []
                for hi in range(2):
                    c = cdpool.tile([64, QW], bf16, tag="cd", name=f"cd{hi}")
                    nc.scalar.copy(c, st["ch"][hi][0:64, :])
                    nc.scalar.copy(st["zrow"][:, hi * QW:(hi + 1) * QW],
                                   st["ch"][hi][64:65, :])
                    st["cd"].append(c)

            def n2():
                zd = drampool.tile([1, 2 * QW], f32, tag="zd", name="zd")
                nc.sync.dma_start(out=zd, in_=st["zrow"])
                zbc = bass.AP(
                    tensor=zd.tensor,
                    offset=zd.offset,
                    ap=[[0, 64]] + list(zd.ap[-1:]),
                )
                rz = zpool.tile([64, 2 * QW], f32, tag="rz", name="rz")
                nc.sync.dma_start(out=rz, in_=zbc)
                nc.vector.reciprocal_approx_fast(rz, rz)
                st["rz"] = rz

            def n3():
                nc.gpsimd.tensor_tensor(
                    attn2[0:64, p, cols], st["cd"][0], st["rz"][:, 0:QW],
                    ALU.mult,
                )
                atmp = zpool.tile([64, QW], bf16, tag="atmp", name="atmp")
                nc.gpsimd.tensor_tensor(atmp, st["cd"][1], st["rz"][:, QW:2 * QW],
                                        ALU.mult)
                nc.sync.dma_start(out=attn2[64:128, p, cols], in_=atmp)

            lams.append(("n1", (p, q), n1))
            lams.append(("n2", (p, q), n2))
            lams.append(("n3", (p, q), n3))
            return lams

        # ---- schedule -------------------------------------------------
        # upfront: Q projection for quarter 0, both pairs
        for _, _, lam in qq_lams(0, 0) + qq_lams(1, 0):
            lam()

        # V-proj filler lambdas, interleaved f1/f2 so <=2 vps psum live
        vfill = []
        prev = None
        for t in range(nt):
            f1, f2 = v_lams(t)
            vfill.append(f1)
            if prev is not None:
                vfill.append(prev)
            prev = f2
        vfill.append(prev)

        carry = list(qq_lams(0, 1) + qq_lams(1, 1))
        own = attn_round(0, 0, (), carry, fill_start=4)
        carry = carry + vfill + own
        for r in range(1, 2 * nq):
            p, q = r % 2, r // 2
            if p == 1 and q + 1 < nq:
                carry = qq_lams(0, q + 1) + qq_lams(1, q + 1) + carry
            rdve = dve_t if r >= 2 else ()
            own = attn_round(p, q, rdve, carry)
            carry = carry + own
            if p == 1:
                carry = carry + burst_lams(q)
        for _, _, lam in carry:
            lam()
        if debug_taps:
            dq2 = nc.dram_tensor("dbg_q2", [P, 2, s], bf16, kind="ExternalOutput").ap()
            nc.sync.dma_start(out=dq2, in_=q2)
            dk2 = nc.dram_tensor("dbg_k2", [P, 2, s], bf16, kind="ExternalOutput").ap()
            nc.sync.dma_start(out=dk2, in_=k2)
            dva = nc.dram_tensor("dbg_vaug", [P, NH, nt, 65], bf16, kind="ExternalOutput").ap()
            nc.sync.dma_start(out=dva, in_=v_aug)
            dat = nc.dram_tensor("dbg_attn", [P, 2, s], bf16, kind="ExternalOutput").ap()
            nc.sync.dma_start(out=dat, in_=attn2)

    nc.compile()
    return nc


def get_bass(s=S):
    if s not in _BUILD_CACHE:
        _BUILD_CACHE[s] = build_bass(s)
    return _BUILD_CACHE[s]


def make_in_maps(query, key, value, Wq, bq, Wk, bk, Wv, bv, Wo):
    """Host-side sharding: per-core input dict for core = b*4 + g."""
    in_maps = []
    for core in range(8):
        b, g = core // 4, core % 4
        cs = slice(g * 256, (g + 1) * 256)
        # pair-packed: wo_h[hd + 64*(h%2), h//2, :] = Wo[:, g*256 + h*64 + hd]
        wo_h = (
            np.ascontiguousarray(Wo[:, cs].T)
            .reshape(2, P, D)
            .transpose(1, 0, 2)
        )
        m = {
            "xq_t": np.ascontiguousarray(query[:, b, :].T).astype(BF16),
            "xk_t": np.ascontiguousarray(key[:, b, :].T).astype(BF16),
            "xv_t": np.ascontiguousarray(value[:, b, :].T).astype(BF16),
            "wq_t": np.ascontiguousarray(Wq[cs, :].T).astype(BF16),
            "wk_t": np.ascontiguousarray(Wk[cs, :].T).astype(BF16),
            "wv_t": np.ascontiguousarray(Wv[cs, :].T).astype(BF16),
            "wo_h": np.ascontiguousarray(wo_h).astype(BF16),
            "bq2": np.ascontiguousarray(bq[cs].reshape(2, P).T).astype(np.float32),
            "bk2": np.ascontiguousarray(bk[cs].reshape(2, P).T).astype(np.float32),
            "bv4": np.ascontiguousarray(
                np.broadcast_to(bv[cs], (P, 256))
            ).astype(np.float32),
        }
        in_maps.append(m)
    return in_maps


def kernel(query, key, value, Wq, bq, Wk, bk, Wv, bv, Wo, bo):
    from concourse.bass_utils import run_bass_kernel_spmd

    query = np.asarray(query, dtype=np.float32)
    key = np.asarray(key, dtype=np.float32)
    value = np.asarray(value, dtype=np.float32)
    Wq = np.asarray(Wq, dtype=np.float32)
    Wk = np.asarray(Wk, dtype=np.float32)
    Wv = np.asarray(Wv, dtype=np.float32)
    Wo = np.asarray(Wo, dtype=np.float32)

    nc = get_bass(S)
    in_maps = make_in_maps(query, key, value, Wq, bq, Wk, bk, Wv, bv, Wo)
    res = run_bass_kernel_spmd(nc, in_maps, core_ids=list(range(8)))
    outs = [res.results[c]["out"] for c in range(8)]

    full = np.empty((S, B, D), dtype=np.float32)
    bo32 = np.asarray(bo, dtype=np.float32)
    for b in range(B):
        acc = outs[b * 4].astype(np.float32).copy()
        for g in range(1, 4):
            acc += outs[b * 4 + g]
        full[:, b, :] = acc + bo32[None, :]
    return full
